# revision 19
# baseline (speedup 1.0000x reference)
"""AnchorBankCAA fused segment-mean/EMA/loss kernel for 8 TRN2 NeuronCores.

Strategy (data-parallel over B, rows domain-sorted host-side):
  - host sorts rows by domain and packs them into single-domain groups of
    3072 rows (24 tiles of 128), padded with inert rows (mu=0, y=999);
    22 groups per core (67584 rows, +3.1% padding)
  - mu ships as fp16 with per-tile layout [mu | mu^2-slot]; ACT/GpSimd
    alternate computing the squares into the slot
  - per tile: ONE matmul — class one-hot (is_equal vs iota) as stationary,
    [mu | mu^2] (128, 512) moving — accumulating [feature sums | sqsums]
    per class into a ping-pong PSUM stage bank
  - per group: 6 masked-identity matmuls flush the stage into 6 per-domain
    PSUM accumulators (mask = host-provided group-domain one-hot)
  - AllReduce the (128, 6*512) f32 partials, then a replicated final phase
    (EMA + CAA/stats losses) computes the outputs; counts come from a host
    bincount (index metadata only)
"""
import sys

sys.path.insert(0, "/opt/trn_rl_repo")

import numpy as np
from concourse import bacc, mybir
from concourse.alu_op_type import AluOpType
from concourse.tile import TileContext
from concourse.bass_utils import run_bass_kernel_spmd

C = 128          # classes
ND = 6           # domains
D = 256          # feat dim
B = 524288
NCORES = 8
P = 128
GT = 24          # tiles per group
GR = GT * P      # rows per group (3072)
NG = 22          # groups per core
NTp = NG * GT    # tiles per core (528)
R = NTp * P      # padded rows per core (67584)
MOM = 0.9
W = 2 * D        # 512: [sums | sqsums] stage width
CCN = P * ND * W  # AllReduce payload floats

f32 = mybir.dt.float32
f16 = mybir.dt.float16
RG = [list(range(NCORES))]

_compiled = None


def _build():
    nc = bacc.Bacc(num_devices=NCORES)

    f8 = mybir.dt.float8e4
    mu = nc.dram_tensor("mu", (R, D), f8, kind="ExternalInput")
    # pair one-hots: per 256-row pair a (128, 2, 128) fp8 block
    ohp = nc.dram_tensor("ohp", (P, NTp * C), f8, kind="ExternalInput")
    mids_d = nc.dram_tensor("mids", (P, (NG // 2) * ND * 2 * C),
                            mybir.dt.float8e4, kind="ExternalInput")
    invc = nc.dram_tensor("invc", (P, ND * D), f32, kind="ExternalInput")
    has01 = nc.dram_tensor("has01", (P, ND * D), f32, kind="ExternalInput")
    cnts = nc.dram_tensor("cnts", (P, ND), f32, kind="ExternalInput")
    dcnt = nc.dram_tensor("dcnt", (ND, 1), f32, kind="ExternalInput")
    anchors = nc.dram_tensor("anchors", (ND, C, D), f32, kind="ExternalInput")
    dmeans = nc.dram_tensor("dmeans", (ND, D), f32, kind="ExternalInput")
    dvars = nc.dram_tensor("dvars", (ND, D), f32, kind="ExternalInput")

    o_anch = nc.dram_tensor("o_anch", (ND, C, D), f32, kind="ExternalOutput")
    o_means = nc.dram_tensor("o_means", (ND, D), f32, kind="ExternalOutput")
    o_vars = nc.dram_tensor("o_vars", (ND, D), f32, kind="ExternalOutput")
    o_loss = nc.dram_tensor("o_loss", (1, 1), f32, kind="ExternalOutput")

    dst_scr = nc.dram_tensor("dst_scr", (ND, W), f32, kind="Internal")
    bf16 = mybir.dt.bfloat16
    cc_in = nc.dram_tensor("cc_in", (CCN,), bf16, kind="Internal")
    cc_out = nc.dram_tensor("cc_out", (CCN,), bf16, kind="Internal",
                            addr_space="Shared")

    iota128_d = nc.inline_tensor(
        np.tile(np.arange(C, dtype=np.float16), (P, 1)), "iota128")
    ident16_d = nc.inline_tensor(np.eye(P, dtype=np.float16), "ident16")
    ident_d = nc.inline_tensor(np.eye(P, dtype=np.float32), "ident")
    offdiag_d = nc.inline_tensor(
        (1.0 - np.eye(C, dtype=np.float32)), "offdiag")

    with TileContext(nc) as tc:
        with (
            tc.tile_pool(name="singles", bufs=1) as sg,
            tc.tile_pool(name="grp", bufs=3) as grp,
            tc.tile_pool(name="work", bufs=2) as wp,
        ):


            accctx = tc.tile_pool(name="acc", bufs=1, space="PSUM")
            pacc = accctx.__enter__()
            stage = [pacc.tile([P, W], f32, tag=f"stage{k}",
                               name=f"stage{k}") for k in range(2)]
            finals = [pacc.tile([P, W], f32, tag=f"fin{d}",
                                name=f"fin{d}") for d in range(ND)]

            # dram view: mu rows host-ordered (g, p, u) -> contiguous
            # 6 KiB per partition per group
            muv = mu.ap().rearrange("(g p u) f -> g p (u f)", p=P, u=GT)
            ohv = ohp.ap().rearrange("p (g j) -> g p j", g=NG)
            NPAIR = GT // 2
            MD = GT * D      # 6144: offset of the squares half
            # square engine split (multi-tile ops amortize fixed cost)
            SQRUNS = [(0, 12, "act"), (12, 21, "dve"), (21, 24, "gp")]
            for g in range(NG):
                gt = grp.tile([P, GT * W], f8, name="gt", tag="gt")
                nc.sync.dma_start(gt[:, 0:MD], muv[g])
                ohg = wp.tile([P, GT * C], f8, tag="ohg", name="ohg", bufs=3)
                nc.sync.dma_start(ohg[:], ohv[g])
                if g % 2 == 0:
                    mid_g = wp.tile([P, ND * 2 * C], f8, tag="midg",
                                    name="midg", bufs=3)
                    nc.sync.dma_start(
                        mid_g[:],
                        mids_d.ap()[:, (g // 2) * ND * 2 * C:
                                    (g // 2 + 1) * ND * 2 * C])
                    stAB = wp.tile([P, 2 * W], f8, tag="stAB", name="stAB",
                                   bufs=2)
                stg = stage[g % 2]
                for a, b, eng in SQRUNS:
                    msl = gt[:, a * D:b * D]
                    sqs = gt[:, MD + a * D:MD + b * D]
                    if eng == "act":
                        nc.scalar.square(sqs, msl)
                    elif eng == "gp":
                        nc.gpsimd.tensor_tensor(sqs, msl, msl,
                                                AluOpType.mult)
                    else:
                        nc.vector.tensor_tensor(sqs, msl, msl,
                                                AluOpType.mult)
                gtv = gt[:].rearrange("p (h k e w) -> p k e h w",
                                      h=2, k=NPAIR, e=2)
                for k in range(NPAIR):
                    # DoubleRow: one MM covers 256 rows; rhs free order
                    # (e, h, w) flattens to [mu_e0|sq_e0|mu_e1|sq_e1]
                    lw = ohg[:].rearrange("p (k e c) -> p k e c",
                                          k=NPAIR, e=2)[:, k]
                    nc.tensor.matmul(
                        stg[:], lw, gtv[:, k],
                        start=(k == 0), stop=(k == NPAIR - 1),
                        perf_mode=mybir.MatmulPerfMode.DoubleRow)
                # copy this group's stage into its pair slot (fp8)
                nc.vector.tensor_copy(
                    stAB[:, (g % 2) * W:(g % 2 + 1) * W], stg[:])
                if g % 2 == 1:
                    # paired DoubleRow flush: 6 MMs cover both groups
                    midv = mid_g[:].rearrange("p (dd e c) -> p dd e c",
                                              dd=ND, e=2)
                    stv = stAB[:].rearrange("p (e w) -> p e w", e=2)
                    for d in range(ND):
                        nc.tensor.matmul(
                            finals[d][:], midv[:, d], stv,
                            start=(g == 1), stop=(g == NG - 1),
                            perf_mode=mybir.MatmulPerfMode.DoubleRow)

            # ---- pack partials (bf16) and AllReduce ----
            ccb = sg.tile([P, ND * W], mybir.dt.bfloat16)
            for d in range(ND):
                nc.vector.tensor_copy(ccb[:, d * W:(d + 1) * W],
                                      finals[d][:])
            accctx.__exit__(None, None, None)
            pfinctx = tc.tile_pool(name="pfin", bufs=4, space="PSUM")
            pfin = pfinctx.__enter__()

            # loop-independent final-phase inputs: hoisted so they load
            # and akeep computes during the main loop / AllReduce
            ident = sg.tile([P, P], f32)
            nc.sync.dma_start(ident[:], ident_d[:])
            offdiag = sg.tile([C, C], f32)
            nc.sync.dma_start(offdiag[:], offdiag_d[:])
            anch = sg.tile([P, ND * D], f32)
            nc.sync.dma_start(
                anch[:].rearrange("c (a f) -> c a f", a=ND),
                anchors.ap().rearrange("a c f -> c a f"))
            dmns = sg.tile([ND, D], f32)
            nc.sync.dma_start(dmns[:], dmeans.ap())
            dvrs = sg.tile([ND, D], f32)
            nc.sync.dma_start(dvrs[:], dvars.ap())
            cnts_s = sg.tile([P, ND], f32)
            nc.sync.dma_start(cnts_s[:], cnts.ap())
            dcnt_s = sg.tile([ND, 1], f32)
            nc.sync.dma_start(dcnt_s[:], dcnt.ap())
            invc_s = sg.tile([P, ND * D], f32)
            nc.sync.dma_start(invc_s[:], invc.ap())
            has01_s = sg.tile([P, ND * D], f32)
            nc.sync.dma_start(has01_s[:], has01.ap())
            akeep = sg.tile([P, ND * D], f32)
            nc.scalar.activation(akeep[:], anch[:],
                                 mybir.ActivationFunctionType.Copy)
            nc.gpsimd.tensor_tensor(akeep[:], akeep[:], has01_s[:],
                                    AluOpType.mult)
            ones128 = sg.tile([P, 1], f32)
            nc.vector.memset(ones128[:], 1.0)
            ones128b = sg.tile([P, 1], mybir.dt.bfloat16)
            nc.vector.memset(ones128b[:], 1.0)
            ones6 = sg.tile([ND, 1], f32)
            nc.vector.memset(ones6[:], 1.0)
            onesrow = sg.tile([1, C], f32)
            nc.vector.memset(onesrow[:], 1.0)

            nc.sync.dma_start(
                cc_in.ap().rearrange("(p j) -> p j", p=P), ccb[:])
            nc.gpsimd.collective_compute(
                "AllReduce", AluOpType.add, replica_groups=RG,
                ins=[cc_in.ap()], outs=[cc_out.ap()])
            segg_b = sg.tile([P, ND * W], mybir.dt.bfloat16)
            nc.sync.dma_start(
                segg_b[:], cc_out.ap().rearrange("(p j) -> p j", p=P))
            segg = sg.tile([P, ND * W], f32)
            nc.vector.tensor_copy(segg[:], segg_b[:])

            # ---- replicated final phase ----

            # new anchors = segg*(h/max(n,1)) + A*(1-h); the two products
            # run on different engines in parallel
            segv = segg[:].rearrange("c (a w) -> c a w", a=ND)[:, :, 0:D]
            mean_a = sg.tile([P, ND * D], f32)
            nc.vector.tensor_tensor(
                mean_a[:].rearrange("c (a f) -> c a f", a=ND), segv,
                invc_s[:].rearrange("c (a f) -> c a f", a=ND),
                AluOpType.mult)
            newA = sg.tile([P, ND * D], f32)
            nc.vector.tensor_tensor(newA[:], mean_a[:], akeep[:],
                                    AluOpType.add)
            nc.sync.dma_start(
                o_anch.ap().rearrange("a c f -> c a f"),
                newA[:].rearrange("c (a f) -> c a f", a=ND))

            # class mean over domains (= A_mean): tree adds on 2 engines
            cmt1 = wp.tile([P, D], f32, tag="cmt", name="cmt1")
            nc.vector.tensor_tensor(cmt1[:], newA[:, 0:D], newA[:, D:2 * D],
                                    AluOpType.add)
            cmt2 = wp.tile([P, D], f32, tag="cmt2", name="cmt2")
            nc.gpsimd.tensor_tensor(cmt2[:], newA[:, 2 * D:3 * D],
                                    newA[:, 3 * D:4 * D], AluOpType.add)
            cmt3 = wp.tile([P, D], f32, tag="cmt3", name="cmt3")
            nc.vector.tensor_tensor(cmt3[:], newA[:, 4 * D:5 * D],
                                    newA[:, 5 * D:6 * D], AluOpType.add)
            nc.vector.tensor_tensor(cmt1[:], cmt1[:], cmt2[:], AluOpType.add)
            cm = sg.tile([P, D], f32)
            nc.vector.tensor_tensor(cm[:], cmt1[:], cmt3[:], AluOpType.add)
            nc.vector.tensor_scalar(cm[:], cm[:], 1.0 / ND, None,
                                    AluOpType.mult)

            # loss_inter helper: sqp = row sums of cm^2 (also used for
            # loss_intra via the E[A^2] - cm^2 identity)
            sqp = sg.tile([P, 1], f32)
            cm2 = wp.tile([P, D], f32, tag="fD", name="cm2")
            nc.scalar.activation(cm2[:], cm[:],
                                 mybir.ActivationFunctionType.Square,
                                 accum_out=sqp[:])

            # loss_intra = [sum(newA^2) - 6*sum(cm^2)] / (6*128*256)
            liA = sg.tile([P, 1], f32)
            sqscr = wp.tile([P, ND * D], f32, tag="sqbig", name="sqscr", bufs=1)
            nc.scalar.activation(sqscr[:], newA[:],
                                 mybir.ActivationFunctionType.Square,
                                 accum_out=liA[:])
            li = sg.tile([P, 1], f32)
            nc.vector.tensor_scalar(li[:], sqp[:], -float(ND), None,
                                    AluOpType.mult)
            nc.vector.tensor_tensor(li[:], liA[:], li[:], AluOpType.add)
            nc.vector.tensor_scalar(li[:], li[:], 1.0 / (ND * C * D), None,
                                    AluOpType.mult)
            amt = sg.tile([P, D], f32)
            amtn = sg.tile([P, D], f32)
            for k in range(2):
                trp = pfin.tile([P, P], f32, tag="fp", name=f"trp{k}")
                nc.tensor.transpose(trp[:], cm[:, k * P:(k + 1) * P],
                                    ident[:])
                nc.vector.tensor_copy(amt[:, k * P:(k + 1) * P], trp[:])
                nc.vector.tensor_scalar(amtn[:, k * P:(k + 1) * P], trp[:],
                                        -2.0, None, AluOpType.mult)
            sqrp = pfin.tile([1, P], f32, tag="fp", name="sqrp")
            nc.tensor.transpose(sqrp[:], sqp[:], ident[:])
            sqr = sg.tile([1, C], f32)
            nc.vector.tensor_copy(sqr[:], sqrp[:])

            d2p = pfin.tile([P, C], f32, tag="fp", name="d2p")
            nc.tensor.matmul(d2p[:], amt[:, 0:P], amtn[:, 0:P],
                             start=True, stop=False)
            nc.tensor.matmul(d2p[:], amt[:, P:2 * P], amtn[:, P:2 * P],
                             start=False, stop=False)
            nc.tensor.matmul(d2p[:], onesrow[:], sqr[:],
                             start=False, stop=False)
            nc.tensor.matmul(d2p[:], sqr[:], onesrow[:],
                             start=False, stop=True)
            d2s = sg.tile([P, C], f32)
            nc.vector.tensor_scalar(d2s[:], d2p[:], 1e-12, None,
                                    AluOpType.max)
            dst = wp.tile([P, C], f32, tag="fD", name="dst")
            nc.scalar.activation(dst[:], d2s[:],
                                 mybir.ActivationFunctionType.Sqrt)
            rel = wp.tile([P, C], f32, tag="fD2", name="rel")
            nc.scalar.activation(rel[:], dst[:],
                                 mybir.ActivationFunctionType.Relu,
                                 bias=1.0, scale=-1.0)
            nc.vector.tensor_tensor(rel[:], rel[:], offdiag[:],
                                    AluOpType.mult)
            ri = sg.tile([P, 1], f32)
            nc.vector.reduce_sum(ri[:], rel[:], axis=mybir.AxisListType.X)
            nc.vector.tensor_scalar(ri[:], ri[:], 1.0 / (C * (C - 1)), None,
                                    AluOpType.mult)

            # per-domain stats: [d_sum | d_sq] = column sums over classes
            rowall = sg.tile([1, ND * W], f32)
            for d in range(ND):
                pt = pfin.tile([1, W], f32, tag="fp", name=f"pt{d}")
                nc.tensor.matmul(pt[:], ones128b[:],
                                 segg_b[:, d * W:(d + 1) * W],
                                 start=True, stop=True)
                nc.vector.tensor_copy(rowall[:, d * W:(d + 1) * W], pt[:])
            nc.sync.dma_start(
                dst_scr.ap().rearrange("a w -> (a w)")[None, :], rowall[:])
            dsts = sg.tile([ND, W], f32)
            nc.sync.dma_start(dsts[:], dst_scr.ap())
            d_sum = dsts[:, 0:D]
            d_sq = dsts[:, D:W]

            safe = sg.tile([ND, 1], f32)
            nc.vector.tensor_scalar(safe[:], dcnt_s[:], 1.0, None,
                                    AluOpType.max)
            rec6 = sg.tile([ND, 1], f32)
            nc.vector.reciprocal(rec6[:], safe[:])
            b_mean = sg.tile([ND, D], f32)
            nc.vector.tensor_scalar(b_mean[:], d_sum, rec6[:, 0:1], None,
                                    AluOpType.mult)
            bm2 = wp.tile([ND, D], f32, tag="g1", name="bm2")
            nc.scalar.activation(bm2[:], b_mean[:],
                                 mybir.ActivationFunctionType.Square)
            nc.vector.tensor_scalar(bm2[:], bm2[:], safe[:, 0:1], None,
                                    AluOpType.mult)
            b_var = sg.tile([ND, D], f32)
            nc.vector.tensor_tensor(b_var[:], d_sq, bm2[:],
                                    AluOpType.subtract)
            cm1 = sg.tile([ND, 1], f32)
            nc.vector.tensor_scalar(cm1[:], dcnt_s[:], -1.0, 1.0,
                                    AluOpType.add, AluOpType.max)
            recd = sg.tile([ND, 1], f32)
            nc.vector.reciprocal(recd[:], cm1[:])
            nc.vector.tensor_scalar(b_var[:], b_var[:], recd[:, 0:1], None,
                                    AluOpType.mult)
            g01 = sg.tile([ND, 1], f32)
            nc.vector.tensor_scalar(g01[:], dcnt_s[:], 1.0, 1.0 - MOM,
                                    AluOpType.is_gt, AluOpType.mult)

            newM = sg.tile([ND, D], f32)
            nc.vector.tensor_tensor(newM[:], b_mean[:], dmns[:],
                                    AluOpType.subtract)
            nc.vector.tensor_scalar(newM[:], newM[:], g01[:, 0:1], None,
                                    AluOpType.mult)
            nc.vector.tensor_tensor(newM[:], dmns[:], newM[:], AluOpType.add)
            nc.sync.dma_start(o_means.ap(), newM[:])
            newV = sg.tile([ND, D], f32)
            nc.vector.tensor_tensor(newV[:], b_var[:], dvrs[:],
                                    AluOpType.subtract)
            nc.vector.tensor_scalar(newV[:], newV[:], g01[:, 0:1], None,
                                    AluOpType.mult)
            nc.vector.tensor_tensor(newV[:], dvrs[:], newV[:], AluOpType.add)
            nc.sync.dma_start(o_vars.ap(), newV[:])

            def _colmean6(src_ap, nm):
                pt = pfin.tile([1, D], f32, tag="fp", name=f"pt_{nm}")
                nc.tensor.matmul(pt[:], ones6[:], src_ap, start=True,
                                 stop=True)
                out = sg.tile([1, D], f32, tag=nm, name=nm)
                nc.vector.tensor_scalar(out[:], pt[:], 1.0 / ND, None,
                                        AluOpType.mult)
                return out

            gm = _colmean6(newM[:], "gm")
            gv = _colmean6(newV[:], "gv")

            def _spread_loss(x_ap, g_ap, nm):
                x2 = wp.tile([ND, D], f32, tag="g1", name=f"x2_{nm}")
                nc.scalar.activation(x2[:], x_ap,
                                     mybir.ActivationFunctionType.Square)
                m2 = _colmean6(x2[:], f"m2_{nm}")
                g2 = wp.tile([1, D], f32, tag="g2", name=f"g2_{nm}")
                nc.scalar.activation(g2[:], g_ap,
                                     mybir.ActivationFunctionType.Square)
                df = wp.tile([1, D], f32, tag="g4", name=f"df_{nm}")
                nc.vector.tensor_tensor(df[:], m2[:], g2[:],
                                        AluOpType.subtract)
                out = sg.tile([1, 1], f32, tag=nm, name=nm)
                nc.vector.reduce_sum(out[:], df[:], axis=mybir.AxisListType.X)
                nc.vector.tensor_scalar(out[:], out[:], 1.0 / D, None,
                                        AluOpType.mult)
                return out

            l_mean = _spread_loss(newM[:], gm[:], "lmean")
            l_var = _spread_loss(newV[:], gv[:], "lvar")

            # mu_mean / mu_var from global sums
            mmp = pfin.tile([1, W], f32, tag="fp", name="mmp")
            nc.tensor.matmul(mmp[:], ones6[:], dsts[:], start=True, stop=True)
            mu_mean = sg.tile([1, D], f32)
            nc.vector.tensor_scalar(mu_mean[:], mmp[:, 0:D], 1.0 / B, None,
                                    AluOpType.mult)
            mu_sq = sg.tile([1, D], f32)
            nc.vector.tensor_scalar(mu_sq[:], mmp[:, D:W], 1.0 / B, None,
                                    AluOpType.mult)
            mm2 = wp.tile([1, D], f32, tag="g2", name="mm2")
            nc.scalar.activation(mm2[:], mu_mean[:],
                                 mybir.ActivationFunctionType.Square)
            mu_var = sg.tile([1, D], f32)
            nc.vector.tensor_tensor(mu_var[:], mu_sq[:], mm2[:],
                                    AluOpType.subtract)

            def _mse_row(a_ap, b_ap, nm):
                df = wp.tile([1, D], f32, tag="g2", name=f"df_{nm}")
                nc.vector.tensor_tensor(df[:], a_ap, b_ap,
                                        AluOpType.subtract)
                s2 = wp.tile([1, D], f32, tag="g3", name=f"s2_{nm}")
                out = sg.tile([1, 1], f32, tag=nm, name=nm)
                nc.scalar.activation(s2[:], df[:],
                                     mybir.ActivationFunctionType.Square,
                                     accum_out=out[:])
                nc.vector.tensor_scalar(out[:], out[:], 1.0 / D, None,
                                        AluOpType.mult)
                return out

            l_mu_mean = _mse_row(mu_mean[:], gm[:], "lmumean")
            l_mu_var = _mse_row(mu_var[:], gv[:], "lmuvar")

            lossp = pfin.tile([1, 1], f32, tag="fp", name="lossp")
            nc.tensor.matmul(lossp[:], ones128[:], li[:],
                             start=True, stop=False)
            nc.tensor.matmul(lossp[:], ones128[:], ri[:],
                             start=False, stop=False)
            one1 = sg.tile([1, 1], f32)
            nc.vector.memset(one1[:], 1.0)
            nc.vector.tensor_tensor(l_mean[:], l_mean[:], l_var[:],
                                    AluOpType.add)
            nc.vector.tensor_tensor(l_mu_mean[:], l_mu_mean[:], l_mu_var[:],
                                    AluOpType.add)
            nc.vector.tensor_tensor(l_mean[:], l_mean[:], l_mu_mean[:],
                                    AluOpType.add)
            nc.tensor.matmul(lossp[:], one1[:], l_mean[:],
                             start=False, stop=True)
            lout = sg.tile([1, 1], f32)
            nc.vector.tensor_copy(lout[:], lossp[:])
            nc.sync.dma_start(o_loss.ap(), lout[:])
            pfinctx.__exit__(None, None, None)

    nc.compile()
    return nc


def _prep_inputs(mu_tilde, anchors, domain_means, domain_vars, y_true,
                 d_true):
    mu_tilde = np.asarray(mu_tilde, dtype=np.float32)
    anchors = np.ascontiguousarray(np.asarray(anchors, dtype=np.float32))
    domain_means = np.ascontiguousarray(
        np.asarray(domain_means, dtype=np.float32))
    domain_vars = np.ascontiguousarray(
        np.asarray(domain_vars, dtype=np.float32))
    y = np.asarray(y_true).astype(np.int64)
    d = np.asarray(d_true).astype(np.int64)

    import ml_dtypes
    mu8 = mu_tilde.astype(ml_dtypes.float8_e4m3)

    # index metadata: counts + domain-sorted group packing
    seg_cnt = np.bincount(d * C + y, minlength=ND * C).reshape(ND, C)
    cnts = np.ascontiguousarray(seg_cnt.T.astype(np.float32))      # (128, 6)
    dcnt = seg_cnt.sum(axis=1).astype(np.float32).reshape(ND, 1)
    # (128, 6*256) broadcast rows: 1/max(cnt,1) and 0.1*(cnt>0) per (c, d)
    hasr = ((seg_cnt > 0) * (1.0 - MOM)).astype(np.float32)
    invr = (hasr / np.maximum(seg_cnt, 1)).astype(np.float32)      # h/max(n,1)
    invc_bc = np.ascontiguousarray(np.repeat(
        invr.T[:, :, None], D, axis=2).reshape(P, ND * D))
    hm1r = (1.0 - hasr).astype(np.float32)
    has01_bc = np.ascontiguousarray(np.repeat(
        hm1r.T[:, :, None], D, axis=2).reshape(P, ND * D))

    order = np.argsort(d, kind="stable")
    dom_counts = np.bincount(d, minlength=ND)
    # single-domain groups of GR rows, padded with -1
    groups = []   # (domain, idx array of len GR)
    pos = 0
    for dom in range(ND):
        n = int(dom_counts[dom])
        idx = order[pos:pos + n]
        pos += n
        ng = (n + GR - 1) // GR
        padded = np.full(ng * GR, -1, dtype=np.int64)
        padded[:n] = idx
        for k in range(ng):
            groups.append((dom, padded[k * GR:(k + 1) * GR]))
    assert len(groups) <= NCORES * NG, len(groups)
    while len(groups) < NCORES * NG:
        groups.append((-1, np.full(GR, -1, dtype=np.int64)))

    in_maps = []
    for i in range(NCORES):
        gs = groups[i * NG:(i + 1) * NG]
        idxs = np.concatenate([g[1] for g in gs])
        # reorder rows to (g, p, u) so each partition's group-slice is
        # one contiguous 6 KiB chunk
        idxs = idxs.reshape(NG, GT, P).transpose(0, 2, 1).reshape(-1)
        valid = idxs >= 0
        muc = np.zeros((R, D), dtype=ml_dtypes.float8_e4m3)
        muc[valid] = mu8[idxs[valid]]
        yv = np.full(R, C, dtype=np.int64)   # pad class -> all-zero one-hot
        yv[valid] = y[idxs[valid]]
        # one-hot blocks: rows ordered (g, p, u); block layout
        # (P, g, pair, e, c) with e = tile parity within the pair
        ohc = np.zeros((R, C + 1), dtype=ml_dtypes.float8_e4m3)
        ohc[np.arange(R), yv] = 1.0
        ohc = ohc[:, :C].reshape(NG, P, GT // 2, 2, C).transpose(1, 0, 2, 3, 4)
        ohc = np.ascontiguousarray(ohc.reshape(P, NTp * C))
        mids = np.zeros(((NG // 2) * ND * 2, P, C),
                        dtype=ml_dtypes.float8_e4m3)
        eye = np.eye(P, dtype=ml_dtypes.float8_e4m3)
        for gi, (dom, _) in enumerate(gs):
            if dom >= 0:
                mids[(gi // 2) * ND * 2 + dom * 2 + (gi % 2)] = eye
        mids = np.ascontiguousarray(
            mids.transpose(1, 0, 2).reshape(P, (NG // 2) * ND * 2 * C))
        in_maps.append({
            "mu": muc,
            "ohp": ohc,
            "mids": mids,
            "invc": invc_bc,
            "has01": has01_bc,
            "cnts": cnts,
            "dcnt": dcnt,
            "anchors": anchors,
            "dmeans": domain_means,
            "dvars": domain_vars,
        })
    return in_maps


def get_compiled():
    global _compiled
    if _compiled is None:
        _compiled = _build()
    return _compiled


def run(in_maps, **kw):
    nc = get_compiled()
    return run_bass_kernel_spmd(nc, in_maps, core_ids=list(range(NCORES)),
                                **kw)


def kernel(mu_tilde, anchors, domain_means, domain_vars, y_true, d_true):
    in_maps = _prep_inputs(mu_tilde, anchors, domain_means, domain_vars,
                           y_true, d_true)
    res = run(in_maps)
    r0 = res.results[0]
    return (
        r0["o_anch"].astype(np.float32),
        r0["o_means"].astype(np.float32),
        r0["o_vars"].astype(np.float32),
        np.float32(r0["o_loss"].reshape(())),
    )


# revision 21
# speedup vs baseline: 1.1679x; 1.1679x over previous
"""AnchorBankCAA fused segment-mean/EMA/loss kernel for 8 TRN2 NeuronCores.

Strategy (data-parallel over B, rows domain-sorted host-side):
  - host sorts rows by domain into single-domain groups of 3072 rows
    (24 tiles of 128), padded with inert rows (mu=0, one-hot=0);
    22 groups per core (67584 rows, +3.1% padding), rows ordered
    (group, partition, tile) so every DMA is one descriptor/partition
  - mu ships as fp8(e4m3); per group the [all mu | all squares] SBUF
    tile is filled by multi-tile square ops split ACT/DVE/GpSimd
  - per 256-row pair: ONE DoubleRow fp8 matmul (host-built pair one-hot
    stationary, [mu|mu^2] moving) accumulates per-class [sums|sqsums]
    into a ping-pong PSUM stage bank
  - per group-pair: 6 DoubleRow fp8 masked-identity matmuls flush the
    two stages into 6 per-domain PSUM accumulators
  - bf16 AllReduce of the (128, 6*512) partials; replicated final phase
    (EMA + CAA/stats losses) on every core; counts/scale masks come from
    a host bincount (index metadata only)
"""
import sys

sys.path.insert(0, "/opt/trn_rl_repo")

import numpy as np
from concourse import bacc, mybir
from concourse.alu_op_type import AluOpType
from concourse.tile import TileContext
from concourse.bass_utils import run_bass_kernel_spmd

C = 128          # classes
ND = 6           # domains
D = 256          # feat dim
B = 524288
NCORES = 8
P = 128
GT = 24          # tiles per group
GR = GT * P      # rows per group (3072)
NG = 22          # groups per core
NTp = NG * GT    # tiles per core (528)
R = NTp * P      # padded rows per core (67584)
MOM = 0.9
W = 2 * D        # 512: [sums | sqsums] stage width
CCN = P * ND * W  # AllReduce payload floats

f32 = mybir.dt.float32
f16 = mybir.dt.float16
RG = [list(range(NCORES))]

_compiled = None


def _build():
    nc = bacc.Bacc(num_devices=NCORES)

    f8 = mybir.dt.float8e4
    mu = nc.dram_tensor("mu", (R, D), f8, kind="ExternalInput")
    # pair one-hots: per 256-row pair a (128, 2, 128) fp8 block
    ohp = nc.dram_tensor("ohp", (P, NTp * C), f8, kind="ExternalInput")
    mids_d = nc.dram_tensor("mids", (P, (NG // 2) * ND * 2 * C),
                            mybir.dt.float8e4, kind="ExternalInput")
    invc = nc.dram_tensor("invc", (P, ND * D), f32, kind="ExternalInput")
    has01 = nc.dram_tensor("has01", (P, ND * D), f32, kind="ExternalInput")
    cnts = nc.dram_tensor("cnts", (P, ND), f32, kind="ExternalInput")
    dcnt = nc.dram_tensor("dcnt", (ND, 1), f32, kind="ExternalInput")
    anchors = nc.dram_tensor("anchors", (ND, C, D), f32, kind="ExternalInput")
    dmeans = nc.dram_tensor("dmeans", (ND, D), f32, kind="ExternalInput")
    dvars = nc.dram_tensor("dvars", (ND, D), f32, kind="ExternalInput")

    o_anch = nc.dram_tensor("o_anch", (ND, C, D), f32, kind="ExternalOutput")
    o_means = nc.dram_tensor("o_means", (ND, D), f32, kind="ExternalOutput")
    o_vars = nc.dram_tensor("o_vars", (ND, D), f32, kind="ExternalOutput")
    o_loss = nc.dram_tensor("o_loss", (1, 1), f32, kind="ExternalOutput")

    dst_scr = nc.dram_tensor("dst_scr", (ND, W), f32, kind="Internal")
    bf16 = mybir.dt.bfloat16
    cc_in = nc.dram_tensor("cc_in", (CCN,), bf16, kind="Internal")
    cc_out = nc.dram_tensor("cc_out", (CCN,), bf16, kind="Internal",
                            addr_space="Shared")

    iota128_d = nc.inline_tensor(
        np.tile(np.arange(C, dtype=np.float16), (P, 1)), "iota128")
    ident16_d = nc.inline_tensor(np.eye(P, dtype=np.float16), "ident16")
    ident_d = nc.inline_tensor(np.eye(P, dtype=np.float32), "ident")
    offdiag_d = nc.inline_tensor(
        (1.0 - np.eye(C, dtype=np.float32)), "offdiag")

    with TileContext(nc) as tc:
        with (
            tc.tile_pool(name="singles", bufs=1) as sg,
            tc.tile_pool(name="grp", bufs=4) as grp,
            tc.tile_pool(name="work", bufs=2) as wp,
        ):


            accctx = tc.tile_pool(name="acc", bufs=1, space="PSUM")
            pacc = accctx.__enter__()
            stage = [pacc.tile([P, W], f32, tag=f"stage{k}",
                               name=f"stage{k}") for k in range(2)]
            finals = [pacc.tile([P, W], f32, tag=f"fin{d}",
                                name=f"fin{d}") for d in range(ND)]

            # dram view: mu rows host-ordered (g, p, u) -> contiguous
            # 6 KiB per partition per group
            muv = mu.ap().rearrange("(g p u) f -> g p (u f)", p=P, u=GT)
            ohv = ohp.ap().rearrange("p (g j) -> g p j", g=NG)
            NPAIR = GT // 2
            MD = GT * D      # 6144: offset of the squares half
            # square engine split (multi-tile ops amortize fixed cost)
            SQRUNS = [(0, 12, "act"), (12, 21, "dve"), (21, 24, "gp")]
            for g in range(NG):
                gt = grp.tile([P, GT * W], f8, name="gt", tag="gt")
                # two chunks so squares can start at half-transfer
                nc.sync.dma_start(gt[:, 0:MD // 2], muv[g][:, 0:MD // 2])
                nc.sync.dma_start(gt[:, MD // 2:MD], muv[g][:, MD // 2:MD])
                ohg = wp.tile([P, GT * C], f8, tag="ohg", name="ohg", bufs=3)
                nc.sync.dma_start(ohg[:], ohv[g])
                if g % 2 == 0:
                    mid_g = wp.tile([P, ND * 2 * C], f8, tag="midg",
                                    name="midg", bufs=3)
                    nc.sync.dma_start(
                        mid_g[:],
                        mids_d.ap()[:, (g // 2) * ND * 2 * C:
                                    (g // 2 + 1) * ND * 2 * C])
                    stAB = wp.tile([P, 2 * W], f8, tag="stAB", name="stAB",
                                   bufs=2)
                stg = stage[g % 2]
                for a, b, eng in SQRUNS:
                    msl = gt[:, a * D:b * D]
                    sqs = gt[:, MD + a * D:MD + b * D]
                    if eng == "act":
                        nc.scalar.square(sqs, msl)
                    elif eng == "gp":
                        nc.gpsimd.tensor_tensor(sqs, msl, msl,
                                                AluOpType.mult)
                    else:
                        nc.vector.tensor_tensor(sqs, msl, msl,
                                                AluOpType.mult)
                gtv = gt[:].rearrange("p (h k e w) -> p k e h w",
                                      h=2, k=NPAIR, e=2)
                for k in range(NPAIR):
                    # DoubleRow: one MM covers 256 rows; rhs free order
                    # (e, h, w) flattens to [mu_e0|sq_e0|mu_e1|sq_e1]
                    lw = ohg[:].rearrange("p (k e c) -> p k e c",
                                          k=NPAIR, e=2)[:, k]
                    nc.tensor.matmul(
                        stg[:], lw, gtv[:, k],
                        start=(k == 0), stop=(k == NPAIR - 1),
                        perf_mode=mybir.MatmulPerfMode.DoubleRow)
                # copy this group's stage into its pair slot (fp8)
                nc.vector.tensor_copy(
                    stAB[:, (g % 2) * W:(g % 2 + 1) * W], stg[:])
                if g % 2 == 1:
                    # paired DoubleRow flush: 6 MMs cover both groups
                    midv = mid_g[:].rearrange("p (dd e c) -> p dd e c",
                                              dd=ND, e=2)
                    stv = stAB[:].rearrange("p (e w) -> p e w", e=2)
                    for d in range(ND):
                        nc.tensor.matmul(
                            finals[d][:], midv[:, d], stv,
                            start=(g == 1), stop=(g == NG - 1),
                            perf_mode=mybir.MatmulPerfMode.DoubleRow)

            # ---- pack partials (bf16) and AllReduce ----
            ccb = sg.tile([P, ND * W], mybir.dt.bfloat16)
            for d in range(ND):
                nc.vector.tensor_copy(ccb[:, d * W:(d + 1) * W],
                                      finals[d][:])
            accctx.__exit__(None, None, None)
            pfinctx = tc.tile_pool(name="pfin", bufs=4, space="PSUM")
            pfin = pfinctx.__enter__()

            # loop-independent final-phase inputs: hoisted so they load
            # and akeep computes during the main loop / AllReduce
            ident = sg.tile([P, P], f32)
            nc.sync.dma_start(ident[:], ident_d[:])
            offdiag = sg.tile([C, C], f32)
            nc.sync.dma_start(offdiag[:], offdiag_d[:])
            anch = sg.tile([P, ND * D], f32)
            nc.sync.dma_start(
                anch[:].rearrange("c (a f) -> c a f", a=ND),
                anchors.ap().rearrange("a c f -> c a f"))
            dmns = sg.tile([ND, D], f32)
            nc.sync.dma_start(dmns[:], dmeans.ap())
            dvrs = sg.tile([ND, D], f32)
            nc.sync.dma_start(dvrs[:], dvars.ap())
            cnts_s = sg.tile([P, ND], f32)
            nc.sync.dma_start(cnts_s[:], cnts.ap())
            dcnt_s = sg.tile([ND, 1], f32)
            nc.sync.dma_start(dcnt_s[:], dcnt.ap())
            invc_s = sg.tile([P, ND * D], f32)
            nc.sync.dma_start(invc_s[:], invc.ap())
            has01_s = sg.tile([P, ND * D], f32)
            nc.sync.dma_start(has01_s[:], has01.ap())
            akeep = sg.tile([P, ND * D], f32)
            nc.scalar.activation(akeep[:], anch[:],
                                 mybir.ActivationFunctionType.Copy)
            nc.gpsimd.tensor_tensor(akeep[:], akeep[:], has01_s[:],
                                    AluOpType.mult)
            ones128 = sg.tile([P, 1], f32)
            nc.vector.memset(ones128[:], 1.0)
            ones128b = sg.tile([P, 1], mybir.dt.bfloat16)
            nc.vector.memset(ones128b[:], 1.0)
            ones6 = sg.tile([ND, 1], f32)
            nc.vector.memset(ones6[:], 1.0)
            onesrow = sg.tile([1, C], f32)
            nc.vector.memset(onesrow[:], 1.0)

            nc.sync.dma_start(
                cc_in.ap().rearrange("(p j) -> p j", p=P), ccb[:])
            nc.gpsimd.collective_compute(
                "AllReduce", AluOpType.add, replica_groups=RG,
                ins=[cc_in.ap()], outs=[cc_out.ap()])
            segg_b = sg.tile([P, ND * W], mybir.dt.bfloat16)
            nc.sync.dma_start(
                segg_b[:], cc_out.ap().rearrange("(p j) -> p j", p=P))
            segg = sg.tile([P, ND * W], f32)
            nc.vector.tensor_copy(segg[:], segg_b[:])

            # ---- replicated final phase ----

            # new anchors = segg*(h/max(n,1)) + A*(1-h); the two products
            # run on different engines in parallel
            segv = segg[:].rearrange("c (a w) -> c a w", a=ND)[:, :, 0:D]
            mean_a = sg.tile([P, ND * D], f32)
            nc.vector.tensor_tensor(
                mean_a[:].rearrange("c (a f) -> c a f", a=ND), segv,
                invc_s[:].rearrange("c (a f) -> c a f", a=ND),
                AluOpType.mult)
            newA = sg.tile([P, ND * D], f32)
            nc.vector.tensor_tensor(newA[:], mean_a[:], akeep[:],
                                    AluOpType.add)
            nc.sync.dma_start(
                o_anch.ap().rearrange("a c f -> c a f"),
                newA[:].rearrange("c (a f) -> c a f", a=ND))

            # class mean over domains (= A_mean): tree adds on 2 engines
            cmt1 = wp.tile([P, D], f32, tag="cmt", name="cmt1")
            nc.vector.tensor_tensor(cmt1[:], newA[:, 0:D], newA[:, D:2 * D],
                                    AluOpType.add)
            cmt2 = wp.tile([P, D], f32, tag="cmt2", name="cmt2")
            nc.gpsimd.tensor_tensor(cmt2[:], newA[:, 2 * D:3 * D],
                                    newA[:, 3 * D:4 * D], AluOpType.add)
            cmt3 = wp.tile([P, D], f32, tag="cmt3", name="cmt3")
            nc.vector.tensor_tensor(cmt3[:], newA[:, 4 * D:5 * D],
                                    newA[:, 5 * D:6 * D], AluOpType.add)
            nc.vector.tensor_tensor(cmt1[:], cmt1[:], cmt2[:], AluOpType.add)
            cm = sg.tile([P, D], f32)
            nc.vector.tensor_tensor(cm[:], cmt1[:], cmt3[:], AluOpType.add)
            nc.vector.tensor_scalar(cm[:], cm[:], 1.0 / ND, None,
                                    AluOpType.mult)

            # loss_inter helper: sqp = row sums of cm^2 (also used for
            # loss_intra via the E[A^2] - cm^2 identity)
            sqp = sg.tile([P, 1], f32)
            cm2 = wp.tile([P, D], f32, tag="fD", name="cm2")
            nc.scalar.activation(cm2[:], cm[:],
                                 mybir.ActivationFunctionType.Square,
                                 accum_out=sqp[:])

            # loss_intra = [sum(newA^2) - 6*sum(cm^2)] / (6*128*256)
            liA = sg.tile([P, 1], f32)
            sqscr = wp.tile([P, ND * D], f32, tag="sqbig", name="sqscr", bufs=1)
            nc.scalar.activation(sqscr[:], newA[:],
                                 mybir.ActivationFunctionType.Square,
                                 accum_out=liA[:])
            li = sg.tile([P, 1], f32)
            nc.vector.tensor_scalar(li[:], sqp[:], -float(ND), None,
                                    AluOpType.mult)
            nc.vector.tensor_tensor(li[:], liA[:], li[:], AluOpType.add)
            nc.vector.tensor_scalar(li[:], li[:], 1.0 / (ND * C * D), None,
                                    AluOpType.mult)
            amt = sg.tile([P, D], f32)
            amtn = sg.tile([P, D], f32)
            for k in range(2):
                trp = pfin.tile([P, P], f32, tag="fp", name=f"trp{k}")
                nc.tensor.transpose(trp[:], cm[:, k * P:(k + 1) * P],
                                    ident[:])
                nc.vector.tensor_copy(amt[:, k * P:(k + 1) * P], trp[:])
                nc.vector.tensor_scalar(amtn[:, k * P:(k + 1) * P], trp[:],
                                        -2.0, None, AluOpType.mult)
            sqrp = pfin.tile([1, P], f32, tag="fp", name="sqrp")
            nc.tensor.transpose(sqrp[:], sqp[:], ident[:])
            sqr = sg.tile([1, C], f32)
            nc.vector.tensor_copy(sqr[:], sqrp[:])

            d2p = pfin.tile([P, C], f32, tag="fp", name="d2p")
            nc.tensor.matmul(d2p[:], amt[:, 0:P], amtn[:, 0:P],
                             start=True, stop=False)
            nc.tensor.matmul(d2p[:], amt[:, P:2 * P], amtn[:, P:2 * P],
                             start=False, stop=False)
            nc.tensor.matmul(d2p[:], onesrow[:], sqr[:],
                             start=False, stop=False)
            nc.tensor.matmul(d2p[:], sqr[:], onesrow[:],
                             start=False, stop=True)
            d2s = sg.tile([P, C], f32)
            nc.vector.tensor_scalar(d2s[:], d2p[:], 1e-12, None,
                                    AluOpType.max)
            dst = wp.tile([P, C], f32, tag="fD", name="dst")
            nc.scalar.activation(dst[:], d2s[:],
                                 mybir.ActivationFunctionType.Sqrt)
            rel = wp.tile([P, C], f32, tag="fD2", name="rel")
            nc.scalar.activation(rel[:], dst[:],
                                 mybir.ActivationFunctionType.Relu,
                                 bias=1.0, scale=-1.0)
            nc.vector.tensor_tensor(rel[:], rel[:], offdiag[:],
                                    AluOpType.mult)
            ri = sg.tile([P, 1], f32)
            nc.vector.reduce_sum(ri[:], rel[:], axis=mybir.AxisListType.X)
            nc.vector.tensor_scalar(ri[:], ri[:], 1.0 / (C * (C - 1)), None,
                                    AluOpType.mult)

            # per-domain stats: [d_sum | d_sq] = column sums over classes
            rowall = sg.tile([1, ND * W], f32)
            for d in range(ND):
                pt = pfin.tile([1, W], f32, tag="fp", name=f"pt{d}")
                nc.tensor.matmul(pt[:], ones128b[:],
                                 segg_b[:, d * W:(d + 1) * W],
                                 start=True, stop=True)
                nc.vector.tensor_copy(rowall[:, d * W:(d + 1) * W], pt[:])
            nc.sync.dma_start(
                dst_scr.ap().rearrange("a w -> (a w)")[None, :], rowall[:])
            dsts = sg.tile([ND, W], f32)
            nc.sync.dma_start(dsts[:], dst_scr.ap())
            d_sum = dsts[:, 0:D]
            d_sq = dsts[:, D:W]

            safe = sg.tile([ND, 1], f32)
            nc.vector.tensor_scalar(safe[:], dcnt_s[:], 1.0, None,
                                    AluOpType.max)
            rec6 = sg.tile([ND, 1], f32)
            nc.vector.reciprocal(rec6[:], safe[:])
            b_mean = sg.tile([ND, D], f32)
            nc.vector.tensor_scalar(b_mean[:], d_sum, rec6[:, 0:1], None,
                                    AluOpType.mult)
            bm2 = wp.tile([ND, D], f32, tag="g1", name="bm2")
            nc.scalar.activation(bm2[:], b_mean[:],
                                 mybir.ActivationFunctionType.Square)
            nc.vector.tensor_scalar(bm2[:], bm2[:], safe[:, 0:1], None,
                                    AluOpType.mult)
            b_var = sg.tile([ND, D], f32)
            nc.vector.tensor_tensor(b_var[:], d_sq, bm2[:],
                                    AluOpType.subtract)
            cm1 = sg.tile([ND, 1], f32)
            nc.vector.tensor_scalar(cm1[:], dcnt_s[:], -1.0, 1.0,
                                    AluOpType.add, AluOpType.max)
            recd = sg.tile([ND, 1], f32)
            nc.vector.reciprocal(recd[:], cm1[:])
            nc.vector.tensor_scalar(b_var[:], b_var[:], recd[:, 0:1], None,
                                    AluOpType.mult)
            g01 = sg.tile([ND, 1], f32)
            nc.vector.tensor_scalar(g01[:], dcnt_s[:], 1.0, 1.0 - MOM,
                                    AluOpType.is_gt, AluOpType.mult)

            newM = sg.tile([ND, D], f32)
            nc.vector.tensor_tensor(newM[:], b_mean[:], dmns[:],
                                    AluOpType.subtract)
            nc.vector.tensor_scalar(newM[:], newM[:], g01[:, 0:1], None,
                                    AluOpType.mult)
            nc.vector.tensor_tensor(newM[:], dmns[:], newM[:], AluOpType.add)
            nc.sync.dma_start(o_means.ap(), newM[:])
            newV = sg.tile([ND, D], f32)
            nc.vector.tensor_tensor(newV[:], b_var[:], dvrs[:],
                                    AluOpType.subtract)
            nc.vector.tensor_scalar(newV[:], newV[:], g01[:, 0:1], None,
                                    AluOpType.mult)
            nc.vector.tensor_tensor(newV[:], dvrs[:], newV[:], AluOpType.add)
            nc.sync.dma_start(o_vars.ap(), newV[:])

            def _colmean6(src_ap, nm):
                pt = pfin.tile([1, D], f32, tag="fp", name=f"pt_{nm}")
                nc.tensor.matmul(pt[:], ones6[:], src_ap, start=True,
                                 stop=True)
                out = sg.tile([1, D], f32, tag=nm, name=nm)
                nc.vector.tensor_scalar(out[:], pt[:], 1.0 / ND, None,
                                        AluOpType.mult)
                return out

            gm = _colmean6(newM[:], "gm")
            gv = _colmean6(newV[:], "gv")

            def _spread_loss(x_ap, g_ap, nm):
                x2 = wp.tile([ND, D], f32, tag="g1", name=f"x2_{nm}")
                nc.scalar.activation(x2[:], x_ap,
                                     mybir.ActivationFunctionType.Square)
                m2 = _colmean6(x2[:], f"m2_{nm}")
                g2 = wp.tile([1, D], f32, tag="g2", name=f"g2_{nm}")
                nc.scalar.activation(g2[:], g_ap,
                                     mybir.ActivationFunctionType.Square)
                df = wp.tile([1, D], f32, tag="g4", name=f"df_{nm}")
                nc.vector.tensor_tensor(df[:], m2[:], g2[:],
                                        AluOpType.subtract)
                out = sg.tile([1, 1], f32, tag=nm, name=nm)
                nc.vector.reduce_sum(out[:], df[:], axis=mybir.AxisListType.X)
                nc.vector.tensor_scalar(out[:], out[:], 1.0 / D, None,
                                        AluOpType.mult)
                return out

            l_mean = _spread_loss(newM[:], gm[:], "lmean")
            l_var = _spread_loss(newV[:], gv[:], "lvar")

            # mu_mean / mu_var from global sums
            mmp = pfin.tile([1, W], f32, tag="fp", name="mmp")
            nc.tensor.matmul(mmp[:], ones6[:], dsts[:], start=True, stop=True)
            mu_mean = sg.tile([1, D], f32)
            nc.vector.tensor_scalar(mu_mean[:], mmp[:, 0:D], 1.0 / B, None,
                                    AluOpType.mult)
            mu_sq = sg.tile([1, D], f32)
            nc.vector.tensor_scalar(mu_sq[:], mmp[:, D:W], 1.0 / B, None,
                                    AluOpType.mult)
            mm2 = wp.tile([1, D], f32, tag="g2", name="mm2")
            nc.scalar.activation(mm2[:], mu_mean[:],
                                 mybir.ActivationFunctionType.Square)
            mu_var = sg.tile([1, D], f32)
            nc.vector.tensor_tensor(mu_var[:], mu_sq[:], mm2[:],
                                    AluOpType.subtract)

            def _mse_row(a_ap, b_ap, nm):
                df = wp.tile([1, D], f32, tag="g2", name=f"df_{nm}")
                nc.vector.tensor_tensor(df[:], a_ap, b_ap,
                                        AluOpType.subtract)
                s2 = wp.tile([1, D], f32, tag="g3", name=f"s2_{nm}")
                out = sg.tile([1, 1], f32, tag=nm, name=nm)
                nc.scalar.activation(s2[:], df[:],
                                     mybir.ActivationFunctionType.Square,
                                     accum_out=out[:])
                nc.vector.tensor_scalar(out[:], out[:], 1.0 / D, None,
                                        AluOpType.mult)
                return out

            l_mu_mean = _mse_row(mu_mean[:], gm[:], "lmumean")
            l_mu_var = _mse_row(mu_var[:], gv[:], "lmuvar")

            lossp = pfin.tile([1, 1], f32, tag="fp", name="lossp")
            nc.tensor.matmul(lossp[:], ones128[:], li[:],
                             start=True, stop=False)
            nc.tensor.matmul(lossp[:], ones128[:], ri[:],
                             start=False, stop=False)
            one1 = sg.tile([1, 1], f32)
            nc.vector.memset(one1[:], 1.0)
            nc.vector.tensor_tensor(l_mean[:], l_mean[:], l_var[:],
                                    AluOpType.add)
            nc.vector.tensor_tensor(l_mu_mean[:], l_mu_mean[:], l_mu_var[:],
                                    AluOpType.add)
            nc.vector.tensor_tensor(l_mean[:], l_mean[:], l_mu_mean[:],
                                    AluOpType.add)
            nc.tensor.matmul(lossp[:], one1[:], l_mean[:],
                             start=False, stop=True)
            lout = sg.tile([1, 1], f32)
            nc.vector.tensor_copy(lout[:], lossp[:])
            nc.sync.dma_start(o_loss.ap(), lout[:])
            pfinctx.__exit__(None, None, None)

    nc.compile()
    return nc


def _prep_inputs(mu_tilde, anchors, domain_means, domain_vars, y_true,
                 d_true):
    mu_tilde = np.asarray(mu_tilde, dtype=np.float32)
    anchors = np.ascontiguousarray(np.asarray(anchors, dtype=np.float32))
    domain_means = np.ascontiguousarray(
        np.asarray(domain_means, dtype=np.float32))
    domain_vars = np.ascontiguousarray(
        np.asarray(domain_vars, dtype=np.float32))
    y = np.asarray(y_true).astype(np.int64)
    d = np.asarray(d_true).astype(np.int64)

    import ml_dtypes
    mu8 = mu_tilde.astype(ml_dtypes.float8_e4m3)

    # index metadata: counts + domain-sorted group packing
    seg_cnt = np.bincount(d * C + y, minlength=ND * C).reshape(ND, C)
    cnts = np.ascontiguousarray(seg_cnt.T.astype(np.float32))      # (128, 6)
    dcnt = seg_cnt.sum(axis=1).astype(np.float32).reshape(ND, 1)
    # (128, 6*256) broadcast rows: 1/max(cnt,1) and 0.1*(cnt>0) per (c, d)
    hasr = ((seg_cnt > 0) * (1.0 - MOM)).astype(np.float32)
    invr = (hasr / np.maximum(seg_cnt, 1)).astype(np.float32)      # h/max(n,1)
    invc_bc = np.ascontiguousarray(np.repeat(
        invr.T[:, :, None], D, axis=2).reshape(P, ND * D))
    hm1r = (1.0 - hasr).astype(np.float32)
    has01_bc = np.ascontiguousarray(np.repeat(
        hm1r.T[:, :, None], D, axis=2).reshape(P, ND * D))

    order = np.argsort(d, kind="stable")
    dom_counts = np.bincount(d, minlength=ND)
    # single-domain groups of GR rows, padded with -1
    groups = []   # (domain, idx array of len GR)
    pos = 0
    for dom in range(ND):
        n = int(dom_counts[dom])
        idx = order[pos:pos + n]
        pos += n
        ng = (n + GR - 1) // GR
        padded = np.full(ng * GR, -1, dtype=np.int64)
        padded[:n] = idx
        for k in range(ng):
            groups.append((dom, padded[k * GR:(k + 1) * GR]))
    assert len(groups) <= NCORES * NG, len(groups)
    while len(groups) < NCORES * NG:
        groups.append((-1, np.full(GR, -1, dtype=np.int64)))

    in_maps = []
    for i in range(NCORES):
        gs = groups[i * NG:(i + 1) * NG]
        idxs = np.concatenate([g[1] for g in gs])
        # reorder rows to (g, p, u) so each partition's group-slice is
        # one contiguous 6 KiB chunk
        idxs = idxs.reshape(NG, GT, P).transpose(0, 2, 1).reshape(-1)
        valid = idxs >= 0
        muc = np.zeros((R, D), dtype=ml_dtypes.float8_e4m3)
        muc[valid] = mu8[idxs[valid]]
        yv = np.full(R, C, dtype=np.int64)   # pad class -> all-zero one-hot
        yv[valid] = y[idxs[valid]]
        # one-hot blocks: rows ordered (g, p, u); block layout
        # (P, g, pair, e, c) with e = tile parity within the pair
        ohc = np.zeros((R, C + 1), dtype=ml_dtypes.float8_e4m3)
        ohc[np.arange(R), yv] = 1.0
        ohc = ohc[:, :C].reshape(NG, P, GT // 2, 2, C).transpose(1, 0, 2, 3, 4)
        ohc = np.ascontiguousarray(ohc.reshape(P, NTp * C))
        mids = np.zeros(((NG // 2) * ND * 2, P, C),
                        dtype=ml_dtypes.float8_e4m3)
        eye = np.eye(P, dtype=ml_dtypes.float8_e4m3)
        for gi, (dom, _) in enumerate(gs):
            if dom >= 0:
                mids[(gi // 2) * ND * 2 + dom * 2 + (gi % 2)] = eye
        mids = np.ascontiguousarray(
            mids.transpose(1, 0, 2).reshape(P, (NG // 2) * ND * 2 * C))
        in_maps.append({
            "mu": muc,
            "ohp": ohc,
            "mids": mids,
            "invc": invc_bc,
            "has01": has01_bc,
            "cnts": cnts,
            "dcnt": dcnt,
            "anchors": anchors,
            "dmeans": domain_means,
            "dvars": domain_vars,
        })
    return in_maps


def get_compiled():
    global _compiled
    if _compiled is None:
        _compiled = _build()
    return _compiled


def run(in_maps, **kw):
    nc = get_compiled()
    return run_bass_kernel_spmd(nc, in_maps, core_ids=list(range(NCORES)),
                                **kw)


def kernel(mu_tilde, anchors, domain_means, domain_vars, y_true, d_true):
    in_maps = _prep_inputs(mu_tilde, anchors, domain_means, domain_vars,
                           y_true, d_true)
    res = run(in_maps)
    r0 = res.results[0]
    return (
        r0["o_anch"].astype(np.float32),
        r0["o_means"].astype(np.float32),
        r0["o_vars"].astype(np.float32),
        np.float32(r0["o_loss"].reshape(())),
    )


# revision 23
# speedup vs baseline: 1.1773x; 1.0081x over previous
"""AnchorBankCAA fused segment-mean/EMA/loss kernel for 8 TRN2 NeuronCores.

Strategy (data-parallel over B, rows domain-sorted host-side):
  - host sorts rows by domain into single-domain groups of 3072 rows
    (24 tiles of 128), padded with inert rows (mu=0, one-hot=0);
    22 groups per core (67584 rows, +3.1% padding), rows ordered
    (group, partition, tile) so every DMA is one descriptor/partition
  - mu ships as fp8(e4m3); per group the [all mu | all squares] SBUF
    tile is filled by multi-tile square ops split ACT/DVE/GpSimd
  - per 256-row pair: ONE DoubleRow fp8 matmul (host-built pair one-hot
    stationary, [mu|mu^2] moving) accumulates per-class [sums|sqsums]
    into a ping-pong PSUM stage bank
  - per group-pair: 6 DoubleRow fp8 masked-identity matmuls flush the
    two stages into 6 per-domain PSUM accumulators
  - bf16 AllReduce of the (128, 6*512) partials; replicated final phase
    (EMA + CAA/stats losses) on every core; counts/scale masks come from
    a host bincount (index metadata only)
"""
import sys

sys.path.insert(0, "/opt/trn_rl_repo")

import numpy as np
from concourse import bacc, mybir
from concourse.alu_op_type import AluOpType
from concourse.tile import TileContext
from concourse.bass_utils import run_bass_kernel_spmd

C = 128          # classes
ND = 6           # domains
D = 256          # feat dim
B = 524288
NCORES = 8
P = 128
GT = 24          # tiles per group
GR = GT * P      # rows per group (3072)
NG = 22          # groups per core
NTp = NG * GT    # tiles per core (528)
R = NTp * P      # padded rows per core (67584)
MOM = 0.9
W = 2 * D        # 512: [sums | sqsums] stage width
CCN = P * ND * W  # AllReduce payload floats

f32 = mybir.dt.float32
f16 = mybir.dt.float16
RG = [list(range(NCORES))]

_compiled = None


def _build():
    nc = bacc.Bacc(num_devices=NCORES)

    f8 = mybir.dt.float8e4
    mu = nc.dram_tensor("mu", (R, D), f8, kind="ExternalInput")
    # pair one-hots: per 256-row pair a (128, 2, 128) fp8 block
    ohp = nc.dram_tensor("ohp", (P, NTp * C), f8, kind="ExternalInput")
    mids_d = nc.dram_tensor("mids", (P, (NG // 2) * ND * 2 * C),
                            mybir.dt.float8e4, kind="ExternalInput")
    invc = nc.dram_tensor("invc", (P, ND * D), f32, kind="ExternalInput")
    has01 = nc.dram_tensor("has01", (P, ND * D), f32, kind="ExternalInput")
    cnts = nc.dram_tensor("cnts", (P, ND), f32, kind="ExternalInput")
    dcnt = nc.dram_tensor("dcnt", (ND, 1), f32, kind="ExternalInput")
    dsc = nc.dram_tensor("dsc", (ND, 4), f32, kind="ExternalInput")
    anchors = nc.dram_tensor("anchors", (ND, C, D), f32, kind="ExternalInput")
    dmeans = nc.dram_tensor("dmeans", (ND, D), f32, kind="ExternalInput")
    dvars = nc.dram_tensor("dvars", (ND, D), f32, kind="ExternalInput")

    o_anch = nc.dram_tensor("o_anch", (ND, C, D), f32, kind="ExternalOutput")
    o_means = nc.dram_tensor("o_means", (ND, D), f32, kind="ExternalOutput")
    o_vars = nc.dram_tensor("o_vars", (ND, D), f32, kind="ExternalOutput")
    o_loss = nc.dram_tensor("o_loss", (1, 1), f32, kind="ExternalOutput")

    dst_scr = nc.dram_tensor("dst_scr", (ND, W), f32, kind="Internal")
    bf16 = mybir.dt.bfloat16
    cc_in = nc.dram_tensor("cc_in", (CCN,), bf16, kind="Internal")
    cc_out = nc.dram_tensor("cc_out", (CCN,), bf16, kind="Internal",
                            addr_space="Shared")

    iota128_d = nc.inline_tensor(
        np.tile(np.arange(C, dtype=np.float16), (P, 1)), "iota128")
    ident16_d = nc.inline_tensor(np.eye(P, dtype=np.float16), "ident16")
    ident_d = nc.inline_tensor(np.eye(P, dtype=np.float32), "ident")
    offdiag_d = nc.inline_tensor(
        (1.0 - np.eye(C, dtype=np.float32)), "offdiag")

    with TileContext(nc) as tc:
        with (
            tc.tile_pool(name="singles", bufs=1) as sg,
            tc.tile_pool(name="grp", bufs=4) as grp,
            tc.tile_pool(name="work", bufs=2) as wp,
        ):


            accctx = tc.tile_pool(name="acc", bufs=1, space="PSUM")
            pacc = accctx.__enter__()
            stage = [pacc.tile([P, W], f32, tag=f"stage{k}",
                               name=f"stage{k}") for k in range(2)]
            finals = [pacc.tile([P, W], f32, tag=f"fin{d}",
                                name=f"fin{d}") for d in range(ND)]

            # dram view: mu rows host-ordered (g, p, u) -> contiguous
            # 6 KiB per partition per group
            muv = mu.ap().rearrange("(g p u) f -> g p (u f)", p=P, u=GT)
            ohv = ohp.ap().rearrange("p (g j) -> g p j", g=NG)
            NPAIR = GT // 2
            MD = GT * D      # 6144: offset of the squares half
            # square engine split (multi-tile ops amortize fixed cost)
            SQRUNS = [(0, 12, "act"), (12, 21, "dve"), (21, 24, "gp")]
            for g in range(NG):
                gt = grp.tile([P, GT * W], f8, name="gt", tag="gt")
                # two chunks so squares can start at half-transfer
                nc.sync.dma_start(gt[:, 0:MD // 2], muv[g][:, 0:MD // 2])
                nc.sync.dma_start(gt[:, MD // 2:MD], muv[g][:, MD // 2:MD])
                ohg = wp.tile([P, GT * C], f8, tag="ohg", name="ohg", bufs=3)
                nc.sync.dma_start(ohg[:], ohv[g])
                if g % 2 == 0:
                    mid_g = wp.tile([P, ND * 2 * C], f8, tag="midg",
                                    name="midg", bufs=3)
                    nc.sync.dma_start(
                        mid_g[:],
                        mids_d.ap()[:, (g // 2) * ND * 2 * C:
                                    (g // 2 + 1) * ND * 2 * C])
                    stAB = wp.tile([P, 2 * W], f8, tag="stAB", name="stAB",
                                   bufs=2)
                stg = stage[g % 2]
                for a, b, eng in SQRUNS:
                    msl = gt[:, a * D:b * D]
                    sqs = gt[:, MD + a * D:MD + b * D]
                    if eng == "act":
                        nc.scalar.square(sqs, msl)
                    elif eng == "gp":
                        nc.gpsimd.tensor_tensor(sqs, msl, msl,
                                                AluOpType.mult)
                    else:
                        nc.vector.tensor_tensor(sqs, msl, msl,
                                                AluOpType.mult)
                gtv = gt[:].rearrange("p (h k e w) -> p k e h w",
                                      h=2, k=NPAIR, e=2)
                for k in range(NPAIR):
                    # DoubleRow: one MM covers 256 rows; rhs free order
                    # (e, h, w) flattens to [mu_e0|sq_e0|mu_e1|sq_e1]
                    lw = ohg[:].rearrange("p (k e c) -> p k e c",
                                          k=NPAIR, e=2)[:, k]
                    nc.tensor.matmul(
                        stg[:], lw, gtv[:, k],
                        start=(k == 0), stop=(k == NPAIR - 1),
                        perf_mode=mybir.MatmulPerfMode.DoubleRow)
                # copy this group's stage into its pair slot (fp8)
                nc.vector.tensor_copy(
                    stAB[:, (g % 2) * W:(g % 2 + 1) * W], stg[:])
                if g % 2 == 1:
                    # paired DoubleRow flush: 6 MMs cover both groups
                    midv = mid_g[:].rearrange("p (dd e c) -> p dd e c",
                                              dd=ND, e=2)
                    stv = stAB[:].rearrange("p (e w) -> p e w", e=2)
                    for d in range(ND):
                        nc.tensor.matmul(
                            finals[d][:], midv[:, d], stv,
                            start=(g == 1), stop=(g == NG - 1),
                            perf_mode=mybir.MatmulPerfMode.DoubleRow)

            # ---- pack partials (bf16) and AllReduce ----
            ccb = sg.tile([P, ND * W], mybir.dt.bfloat16)
            for d in range(ND):
                nc.vector.tensor_copy(ccb[:, d * W:(d + 1) * W],
                                      finals[d][:])
            accctx.__exit__(None, None, None)
            pfinctx = tc.tile_pool(name="pfin", bufs=4, space="PSUM")
            pfin = pfinctx.__enter__()

            # loop-independent final-phase inputs: hoisted so they load
            # and akeep computes during the main loop / AllReduce
            ident = sg.tile([P, P], f32)
            nc.sync.dma_start(ident[:], ident_d[:])
            offdiag = sg.tile([C, C], f32)
            nc.sync.dma_start(offdiag[:], offdiag_d[:])
            anch = sg.tile([P, ND * D], f32)
            nc.sync.dma_start(
                anch[:].rearrange("c (a f) -> c a f", a=ND),
                anchors.ap().rearrange("a c f -> c a f"))
            dmns = sg.tile([ND, D], f32)
            nc.sync.dma_start(dmns[:], dmeans.ap())
            dvrs = sg.tile([ND, D], f32)
            nc.sync.dma_start(dvrs[:], dvars.ap())
            cnts_s = sg.tile([P, ND], f32)
            nc.sync.dma_start(cnts_s[:], cnts.ap())
            dcnt_s = sg.tile([ND, 1], f32)
            nc.sync.dma_start(dcnt_s[:], dcnt.ap())
            dsc_s = sg.tile([ND, 4], f32)
            nc.sync.dma_start(dsc_s[:], dsc.ap())
            invc_s = sg.tile([P, ND * D], f32)
            nc.sync.dma_start(invc_s[:], invc.ap())
            has01_s = sg.tile([P, ND * D], f32)
            nc.sync.dma_start(has01_s[:], has01.ap())
            akeep = sg.tile([P, ND * D], f32)
            nc.scalar.activation(akeep[:], anch[:],
                                 mybir.ActivationFunctionType.Copy)
            nc.gpsimd.tensor_tensor(akeep[:], akeep[:], has01_s[:],
                                    AluOpType.mult)
            ones128 = sg.tile([P, 1], f32)
            nc.vector.memset(ones128[:], 1.0)
            ones128b = sg.tile([P, 1], mybir.dt.bfloat16)
            nc.vector.memset(ones128b[:], 1.0)
            ones6 = sg.tile([ND, 1], f32)
            nc.vector.memset(ones6[:], 1.0)
            onesrow = sg.tile([1, C], f32)
            nc.vector.memset(onesrow[:], 1.0)

            nc.sync.dma_start(
                cc_in.ap().rearrange("(p j) -> p j", p=P), ccb[:])
            nc.gpsimd.collective_compute(
                "AllReduce", AluOpType.add, replica_groups=RG,
                ins=[cc_in.ap()], outs=[cc_out.ap()])
            segg_b = sg.tile([P, ND * W], mybir.dt.bfloat16)
            HB = ND * W // 2
            ccv = cc_out.ap().rearrange("(p j) -> p j", p=P)
            nc.sync.dma_start(segg_b[:, 0:HB], ccv[:, 0:HB])
            nc.sync.dma_start(segg_b[:, HB:], ccv[:, HB:])
            segg = sg.tile([P, ND * W], f32)
            nc.vector.tensor_copy(segg[:, 0:HB], segg_b[:, 0:HB])
            nc.vector.tensor_copy(segg[:, HB:], segg_b[:, HB:])

            # ---- replicated final phase ----

            # new anchors = segg*(h/max(n,1)) + A*(1-h); the two products
            # run on different engines in parallel
            segv = segg[:].rearrange("c (a w) -> c a w", a=ND)[:, :, 0:D]
            invcv = invc_s[:].rearrange("c (a f) -> c a f", a=ND)
            mean_a = sg.tile([P, ND * D], f32)
            meanv = mean_a[:].rearrange("c (a f) -> c a f", a=ND)
            newA = sg.tile([P, ND * D], f32)
            HA = ND // 2
            for hh in range(2):
                s_ = slice(hh * HA, (hh + 1) * HA)
                nc.vector.tensor_tensor(meanv[:, s_], segv[:, s_],
                                        invcv[:, s_], AluOpType.mult)
                nc.vector.tensor_tensor(
                    newA[:, hh * HA * D:(hh + 1) * HA * D],
                    mean_a[:, hh * HA * D:(hh + 1) * HA * D],
                    akeep[:, hh * HA * D:(hh + 1) * HA * D], AluOpType.add)
            nc.sync.dma_start(
                o_anch.ap().rearrange("a c f -> c a f"),
                newA[:].rearrange("c (a f) -> c a f", a=ND))

            # class mean over domains (= A_mean): tree adds on 2 engines
            cmt1 = wp.tile([P, D], f32, tag="cmt", name="cmt1")
            nc.vector.tensor_tensor(cmt1[:], newA[:, 0:D], newA[:, D:2 * D],
                                    AluOpType.add)
            cmt2 = wp.tile([P, D], f32, tag="cmt2", name="cmt2")
            nc.gpsimd.tensor_tensor(cmt2[:], newA[:, 2 * D:3 * D],
                                    newA[:, 3 * D:4 * D], AluOpType.add)
            cmt3 = wp.tile([P, D], f32, tag="cmt3", name="cmt3")
            nc.vector.tensor_tensor(cmt3[:], newA[:, 4 * D:5 * D],
                                    newA[:, 5 * D:6 * D], AluOpType.add)
            nc.vector.tensor_tensor(cmt1[:], cmt1[:], cmt2[:], AluOpType.add)
            cm = sg.tile([P, D], f32)
            nc.vector.tensor_tensor(cm[:], cmt1[:], cmt3[:], AluOpType.add)
            nc.vector.tensor_scalar(cm[:], cm[:], 1.0 / ND, None,
                                    AluOpType.mult)

            # loss_inter helper: sqp = row sums of cm^2 (also used for
            # loss_intra via the E[A^2] - cm^2 identity)
            sqp = sg.tile([P, 1], f32)
            cm2 = wp.tile([P, D], f32, tag="fD", name="cm2")
            nc.scalar.activation(cm2[:], cm[:],
                                 mybir.ActivationFunctionType.Square,
                                 accum_out=sqp[:])

            # loss_intra = [sum(newA^2) - 6*sum(cm^2)] / (6*128*256)
            liA = sg.tile([P, 1], f32)
            sqscr = wp.tile([P, ND * D], f32, tag="sqbig", name="sqscr", bufs=1)
            nc.scalar.activation(sqscr[:], newA[:],
                                 mybir.ActivationFunctionType.Square,
                                 accum_out=liA[:])
            li = sg.tile([P, 1], f32)
            nc.vector.tensor_scalar(li[:], sqp[:], -float(ND), None,
                                    AluOpType.mult)
            nc.vector.tensor_tensor(li[:], liA[:], li[:], AluOpType.add)
            nc.vector.tensor_scalar(li[:], li[:], 1.0 / (ND * C * D), None,
                                    AluOpType.mult)
            amt = sg.tile([P, D], f32)
            amtn = sg.tile([P, D], f32)
            for k in range(2):
                trp = pfin.tile([P, P], f32, tag="fp", name=f"trp{k}")
                nc.tensor.transpose(trp[:], cm[:, k * P:(k + 1) * P],
                                    ident[:])
                nc.vector.tensor_copy(amt[:, k * P:(k + 1) * P], trp[:])
                nc.vector.tensor_scalar(amtn[:, k * P:(k + 1) * P], trp[:],
                                        -2.0, None, AluOpType.mult)
            sqrp = pfin.tile([1, P], f32, tag="fp", name="sqrp")
            nc.tensor.transpose(sqrp[:], sqp[:], ident[:])
            sqr = sg.tile([1, C], f32)
            nc.vector.tensor_copy(sqr[:], sqrp[:])

            d2p = pfin.tile([P, C], f32, tag="fp", name="d2p")
            nc.tensor.matmul(d2p[:], amt[:, 0:P], amtn[:, 0:P],
                             start=True, stop=False)
            nc.tensor.matmul(d2p[:], amt[:, P:2 * P], amtn[:, P:2 * P],
                             start=False, stop=False)
            nc.tensor.matmul(d2p[:], onesrow[:], sqr[:],
                             start=False, stop=False)
            nc.tensor.matmul(d2p[:], sqr[:], onesrow[:],
                             start=False, stop=True)
            d2s = sg.tile([P, C], f32)
            nc.vector.tensor_scalar(d2s[:], d2p[:], 1e-12, None,
                                    AluOpType.max)
            dst = wp.tile([P, C], f32, tag="fD", name="dst")
            nc.scalar.activation(dst[:], d2s[:],
                                 mybir.ActivationFunctionType.Sqrt)
            rel = wp.tile([P, C], f32, tag="fD2", name="rel")
            nc.scalar.activation(rel[:], dst[:],
                                 mybir.ActivationFunctionType.Relu,
                                 bias=1.0, scale=-1.0)
            nc.vector.tensor_tensor(rel[:], rel[:], offdiag[:],
                                    AluOpType.mult)
            ri = sg.tile([P, 1], f32)
            nc.vector.reduce_sum(ri[:], rel[:], axis=mybir.AxisListType.X)
            nc.vector.tensor_scalar(ri[:], ri[:], 1.0 / (C * (C - 1)), None,
                                    AluOpType.mult)

            # per-domain stats: [d_sum | d_sq] = column sums over classes
            rowall = sg.tile([1, ND * W], f32)
            for d in range(ND):
                pt = pfin.tile([1, W], f32, tag="fp", name=f"pt{d}")
                nc.tensor.matmul(pt[:], ones128b[:],
                                 segg_b[:, d * W:(d + 1) * W],
                                 start=True, stop=True)
                nc.vector.tensor_copy(rowall[:, d * W:(d + 1) * W], pt[:])
            nc.sync.dma_start(
                dst_scr.ap().rearrange("a w -> (a w)")[None, :], rowall[:])
            dsts = sg.tile([ND, W], f32)
            nc.sync.dma_start(dsts[:], dst_scr.ap())
            d_sum = dsts[:, 0:D]
            d_sq = dsts[:, D:W]

            safe = dsc_s[:, 0:1]
            rec6 = dsc_s[:, 1:2]
            recd = dsc_s[:, 2:3]
            g01 = dsc_s[:, 3:4]
            b_mean = sg.tile([ND, D], f32)
            nc.vector.tensor_scalar(b_mean[:], d_sum, rec6[:, 0:1], None,
                                    AluOpType.mult)
            bm2 = wp.tile([ND, D], f32, tag="g1", name="bm2")
            nc.scalar.activation(bm2[:], b_mean[:],
                                 mybir.ActivationFunctionType.Square)
            nc.vector.tensor_scalar(bm2[:], bm2[:], safe[0:ND, 0:1], None,
                                    AluOpType.mult)
            b_var = sg.tile([ND, D], f32)
            nc.vector.tensor_tensor(b_var[:], d_sq, bm2[:],
                                    AluOpType.subtract)
            nc.vector.tensor_scalar(b_var[:], b_var[:], recd[0:ND, 0:1],
                                    None, AluOpType.mult)

            newM = sg.tile([ND, D], f32)
            nc.vector.tensor_tensor(newM[:], b_mean[:], dmns[:],
                                    AluOpType.subtract)
            nc.vector.tensor_scalar(newM[:], newM[:], g01[0:ND, 0:1], None,
                                    AluOpType.mult)
            nc.vector.tensor_tensor(newM[:], dmns[:], newM[:], AluOpType.add)
            nc.sync.dma_start(o_means.ap(), newM[:])
            newV = sg.tile([ND, D], f32)
            nc.vector.tensor_tensor(newV[:], b_var[:], dvrs[:],
                                    AluOpType.subtract)
            nc.vector.tensor_scalar(newV[:], newV[:], g01[0:ND, 0:1], None,
                                    AluOpType.mult)
            nc.vector.tensor_tensor(newV[:], dvrs[:], newV[:], AluOpType.add)
            nc.sync.dma_start(o_vars.ap(), newV[:])

            def _colmean6(src_ap, nm):
                pt = pfin.tile([1, D], f32, tag="fp", name=f"pt_{nm}")
                nc.tensor.matmul(pt[:], ones6[:], src_ap, start=True,
                                 stop=True)
                out = sg.tile([1, D], f32, tag=nm, name=nm)
                nc.vector.tensor_scalar(out[:], pt[:], 1.0 / ND, None,
                                        AluOpType.mult)
                return out

            gm = _colmean6(newM[:], "gm")
            gv = _colmean6(newV[:], "gv")

            def _spread_loss(x_ap, g_ap, nm):
                x2 = wp.tile([ND, D], f32, tag="g1", name=f"x2_{nm}")
                nc.scalar.activation(x2[:], x_ap,
                                     mybir.ActivationFunctionType.Square)
                m2 = _colmean6(x2[:], f"m2_{nm}")
                g2 = wp.tile([1, D], f32, tag="g2", name=f"g2_{nm}")
                nc.scalar.activation(g2[:], g_ap,
                                     mybir.ActivationFunctionType.Square)
                df = wp.tile([1, D], f32, tag="g4", name=f"df_{nm}")
                nc.vector.tensor_tensor(df[:], m2[:], g2[:],
                                        AluOpType.subtract)
                out = sg.tile([1, 1], f32, tag=nm, name=nm)
                nc.vector.reduce_sum(out[:], df[:], axis=mybir.AxisListType.X)
                nc.vector.tensor_scalar(out[:], out[:], 1.0 / D, None,
                                        AluOpType.mult)
                return out

            l_mean = _spread_loss(newM[:], gm[:], "lmean")
            l_var = _spread_loss(newV[:], gv[:], "lvar")

            # mu_mean / mu_var from global sums
            mmp = pfin.tile([1, W], f32, tag="fp", name="mmp")
            nc.tensor.matmul(mmp[:], ones6[:], dsts[:], start=True, stop=True)
            mu_mean = sg.tile([1, D], f32)
            nc.vector.tensor_scalar(mu_mean[:], mmp[:, 0:D], 1.0 / B, None,
                                    AluOpType.mult)
            mu_sq = sg.tile([1, D], f32)
            nc.vector.tensor_scalar(mu_sq[:], mmp[:, D:W], 1.0 / B, None,
                                    AluOpType.mult)
            mm2 = wp.tile([1, D], f32, tag="g2", name="mm2")
            nc.scalar.activation(mm2[:], mu_mean[:],
                                 mybir.ActivationFunctionType.Square)
            mu_var = sg.tile([1, D], f32)
            nc.vector.tensor_tensor(mu_var[:], mu_sq[:], mm2[:],
                                    AluOpType.subtract)

            def _mse_row(a_ap, b_ap, nm):
                df = wp.tile([1, D], f32, tag="g2", name=f"df_{nm}")
                nc.vector.tensor_tensor(df[:], a_ap, b_ap,
                                        AluOpType.subtract)
                s2 = wp.tile([1, D], f32, tag="g3", name=f"s2_{nm}")
                out = sg.tile([1, 1], f32, tag=nm, name=nm)
                nc.scalar.activation(s2[:], df[:],
                                     mybir.ActivationFunctionType.Square,
                                     accum_out=out[:])
                nc.vector.tensor_scalar(out[:], out[:], 1.0 / D, None,
                                        AluOpType.mult)
                return out

            l_mu_mean = _mse_row(mu_mean[:], gm[:], "lmumean")
            l_mu_var = _mse_row(mu_var[:], gv[:], "lmuvar")

            lossp = pfin.tile([1, 1], f32, tag="fp", name="lossp")
            nc.tensor.matmul(lossp[:], ones128[:], li[:],
                             start=True, stop=False)
            nc.tensor.matmul(lossp[:], ones128[:], ri[:],
                             start=False, stop=False)
            one1 = sg.tile([1, 1], f32)
            nc.vector.memset(one1[:], 1.0)
            nc.vector.tensor_tensor(l_mean[:], l_mean[:], l_var[:],
                                    AluOpType.add)
            nc.vector.tensor_tensor(l_mu_mean[:], l_mu_mean[:], l_mu_var[:],
                                    AluOpType.add)
            nc.vector.tensor_tensor(l_mean[:], l_mean[:], l_mu_mean[:],
                                    AluOpType.add)
            nc.tensor.matmul(lossp[:], one1[:], l_mean[:],
                             start=False, stop=True)
            lout = sg.tile([1, 1], f32)
            nc.vector.tensor_copy(lout[:], lossp[:])
            nc.sync.dma_start(o_loss.ap(), lout[:])
            pfinctx.__exit__(None, None, None)

    nc.compile()
    return nc


def _prep_inputs(mu_tilde, anchors, domain_means, domain_vars, y_true,
                 d_true):
    mu_tilde = np.asarray(mu_tilde, dtype=np.float32)
    anchors = np.ascontiguousarray(np.asarray(anchors, dtype=np.float32))
    domain_means = np.ascontiguousarray(
        np.asarray(domain_means, dtype=np.float32))
    domain_vars = np.ascontiguousarray(
        np.asarray(domain_vars, dtype=np.float32))
    y = np.asarray(y_true).astype(np.int64)
    d = np.asarray(d_true).astype(np.int64)

    import ml_dtypes
    mu8 = mu_tilde.astype(ml_dtypes.float8_e4m3)

    # index metadata: counts + domain-sorted group packing
    seg_cnt = np.bincount(d * C + y, minlength=ND * C).reshape(ND, C)
    cnts = np.ascontiguousarray(seg_cnt.T.astype(np.float32))      # (128, 6)
    dcnt = seg_cnt.sum(axis=1).astype(np.float32).reshape(ND, 1)
    safe_h = np.maximum(dcnt, 1.0)
    dsc_h = np.ascontiguousarray(np.concatenate([
        safe_h, (1.0 / safe_h),
        1.0 / np.maximum(dcnt - 1.0, 1.0),
        (dcnt > 1.0) * (1.0 - MOM)], axis=1).astype(np.float32))
    # (128, 6*256) broadcast rows: 1/max(cnt,1) and 0.1*(cnt>0) per (c, d)
    hasr = ((seg_cnt > 0) * (1.0 - MOM)).astype(np.float32)
    invr = (hasr / np.maximum(seg_cnt, 1)).astype(np.float32)      # h/max(n,1)
    invc_bc = np.ascontiguousarray(np.repeat(
        invr.T[:, :, None], D, axis=2).reshape(P, ND * D))
    hm1r = (1.0 - hasr).astype(np.float32)
    has01_bc = np.ascontiguousarray(np.repeat(
        hm1r.T[:, :, None], D, axis=2).reshape(P, ND * D))

    order = np.argsort(d, kind="stable")
    dom_counts = np.bincount(d, minlength=ND)
    # single-domain groups of GR rows, padded with -1
    groups = []   # (domain, idx array of len GR)
    pos = 0
    for dom in range(ND):
        n = int(dom_counts[dom])
        idx = order[pos:pos + n]
        pos += n
        ng = (n + GR - 1) // GR
        padded = np.full(ng * GR, -1, dtype=np.int64)
        padded[:n] = idx
        for k in range(ng):
            groups.append((dom, padded[k * GR:(k + 1) * GR]))
    assert len(groups) <= NCORES * NG, len(groups)
    while len(groups) < NCORES * NG:
        groups.append((-1, np.full(GR, -1, dtype=np.int64)))

    in_maps = []
    for i in range(NCORES):
        gs = groups[i * NG:(i + 1) * NG]
        idxs = np.concatenate([g[1] for g in gs])
        # reorder rows to (g, p, u) so each partition's group-slice is
        # one contiguous 6 KiB chunk
        idxs = idxs.reshape(NG, GT, P).transpose(0, 2, 1).reshape(-1)
        valid = idxs >= 0
        muc = np.zeros((R, D), dtype=ml_dtypes.float8_e4m3)
        muc[valid] = mu8[idxs[valid]]
        yv = np.full(R, C, dtype=np.int64)   # pad class -> all-zero one-hot
        yv[valid] = y[idxs[valid]]
        # one-hot blocks: rows ordered (g, p, u); block layout
        # (P, g, pair, e, c) with e = tile parity within the pair
        ohc = np.zeros((R, C + 1), dtype=ml_dtypes.float8_e4m3)
        ohc[np.arange(R), yv] = 1.0
        ohc = ohc[:, :C].reshape(NG, P, GT // 2, 2, C).transpose(1, 0, 2, 3, 4)
        ohc = np.ascontiguousarray(ohc.reshape(P, NTp * C))
        mids = np.zeros(((NG // 2) * ND * 2, P, C),
                        dtype=ml_dtypes.float8_e4m3)
        eye = np.eye(P, dtype=ml_dtypes.float8_e4m3)
        for gi, (dom, _) in enumerate(gs):
            if dom >= 0:
                mids[(gi // 2) * ND * 2 + dom * 2 + (gi % 2)] = eye
        mids = np.ascontiguousarray(
            mids.transpose(1, 0, 2).reshape(P, (NG // 2) * ND * 2 * C))
        in_maps.append({
            "mu": muc,
            "ohp": ohc,
            "mids": mids,
            "invc": invc_bc,
            "has01": has01_bc,
            "cnts": cnts,
            "dcnt": dcnt,
            "dsc": dsc_h,
            "anchors": anchors,
            "dmeans": domain_means,
            "dvars": domain_vars,
        })
    return in_maps


def get_compiled():
    global _compiled
    if _compiled is None:
        _compiled = _build()
    return _compiled


def run(in_maps, **kw):
    nc = get_compiled()
    return run_bass_kernel_spmd(nc, in_maps, core_ids=list(range(NCORES)),
                                **kw)


def kernel(mu_tilde, anchors, domain_means, domain_vars, y_true, d_true):
    in_maps = _prep_inputs(mu_tilde, anchors, domain_means, domain_vars,
                           y_true, d_true)
    res = run(in_maps)
    r0 = res.results[0]
    return (
        r0["o_anch"].astype(np.float32),
        r0["o_means"].astype(np.float32),
        r0["o_vars"].astype(np.float32),
        np.float32(r0["o_loss"].reshape(())),
    )
